# revision 2
# baseline (speedup 1.0000x reference)
"""Trainium2 Bass kernel for nn_Kernel_6199962355332830965 (sparse_attention).

Math (validated vs reference in numpy, ~6e-7):
    out = -s * B'^T @ x,  B' = w6 @ G,  G = t5 @ t12^T
    t5  = p5 * min(x, roll_w(x, 1)),  t12 = max(x, t11)
    t11 = 15-tap conv of t1 = max_c(x), folded weights
          K_effT[3j+k, o] = sum_m w8[m,j] * w10[(o-1)%C, 3m+k]

v5 scheduling: xT drains on the scalar engine (vector freed for reduces and
min/mult), queues ordered by expected ready time, t1 DMA chain split across
the SP and ACT queues, G matmuls lag the max chunks by one, dummy
transposes bridge the chain wait to keep the PE clock un-throttled.
"""

import os
import sys

import numpy as np

for _p in ("/opt/trn_rl_repo", "/root/.axon_site/_ro/trn_rl_repo"):
    if os.path.isdir(_p) and _p not in sys.path:
        sys.path.append(_p)

import ml_dtypes

import concourse.bass as bass
import concourse.tile as tile
from concourse import bacc, mybir
from concourse.bass_utils import run_bass_kernel_spmd

N, C, H, W = 16, 128, 48, 48
HW = H * W
NCORES = 8
NB = N // NCORES
SCALE = float(-1.0 / (np.sqrt(np.float32(C)) * np.sqrt(np.float32(HW))))

F32 = mybir.dt.float32
BF16 = mybir.dt.bfloat16
BF16NP = ml_dtypes.bfloat16

CHUNKS512 = [(c0, min(512, HW - c0)) for c0 in range(0, HW, 512)]
NDUMMY = 10


def build_kernel(tc, out_d, x_d, p5_d, w6T_d, keff_d, ident_d):
    nc = tc.nc
    mmax = mybir.AluOpType.max
    mmin = mybir.AluOpType.min
    mmul = mybir.AluOpType.mult

    with (
        tc.tile_pool(name="const", bufs=1) as cpool,
        tc.tile_pool(name="batch", bufs=1) as bpool,
        tc.tile_pool(name="ps_wide", bufs=6, space="PSUM") as ps_wide,
        tc.tile_pool(name="ps_cc", bufs=2, space="PSUM") as ps_cc,
    ):
        # ---- x loads on SP; consts on ACT (ident, p5 first) --------------
        x_l = []
        for b in range(NB):
            xb = bpool.tile([C, HW], BF16, tag=f"x{b}")
            x_l.append(xb)
        for b in range(NB):
            xsrc = x_d.ap()[b]
            nc.sync.dma_start(x_l[b][:, 0:1152], xsrc[:, 0:1152])
            nc.sync.dma_start(x_l[b][:, 1152:HW], xsrc[:, 1152:HW])

        ident_bf = cpool.tile([128, 128], BF16, tag="ident")
        nc.scalar.dma_start(ident_bf[:], ident_d.ap())
        p5_sb = cpool.tile([C, HW], BF16, tag="p5sb")
        nc.scalar.dma_start(p5_sb[:], p5_d.ap())
        K_effT = cpool.tile([15, C], BF16, tag="KeffT")
        nc.scalar.dma_start(K_effT[:], keff_d.ap())
        w6T = cpool.tile([C, C], BF16, tag="w6T")
        nc.scalar.dma_start(w6T[:], w6T_d.ap())

        t1p3_l, t1flat_l, shifted_l = [], [], []
        for b in range(NB):
            t1p3 = cpool.tile([3, 48 * 60], BF16, tag=f"t1p3{b}")
            nc.gpsimd.memset(t1p3[:], 0.0)
            t1p3_l.append(t1p3)
            t1flat = cpool.tile([1, 96 + HW + 96], BF16, tag=f"t1flat{b}")
            nc.gpsimd.memset(t1flat[:], 0.0)
            t1flat_l.append(t1flat)
            sh = cpool.tile([15, HW], BF16, tag=f"shift{b}")
            shifted_l.append(sh)

        def k_dma(eng, b, k):
            tf = t1flat_l[b][:]
            ksrc = bass.AP(tf.tensor, tf.offset + 96 * k, [list(tf.ap[0]), [1, HW]])
            kdst = t1p3_l[b][:].rearrange("k (r c) -> k r c", c=60)[
                k : k + 1, 0:48, 6:54
            ]
            eng.dma_start(kdst.opt(), ksrc)

        def j_dma(eng, b, j):
            t1p3v = t1p3_l[b][:].rearrange("k (r c) -> k r c", c=60)
            eng.dma_start(
                shifted_l[b][3 * j : 3 * j + 3, :],
                t1p3v[:, 0:48, 3 * j : 3 * j + 48],
            )

        def emit_min(b, tmin):
            x3 = x_l[b][:].rearrange("c (h w) -> c h w", w=W)
            tm3 = tmin[:].rearrange("c (h w) -> c h w", w=W)
            nc.vector.tensor_tensor(
                tm3[:, :, 1:W], x3[:, :, 1:W], x3[:, :, 0 : W - 1], mmin
            )
            nc.vector.tensor_tensor(
                tm3[:, :, 0:1], x3[:, :, 0:1], x3[:, :, W - 1 : W], mmin
            )

        # ---- xT transposes (PE), drains (S), reduces (V, from PSUM) ------
        xT_l, t1pk_l, t1row_l = [], [], []
        tmin_l = [None, None]
        for b in range(NB):
            xT = bpool.tile([C, HW], BF16, tag=f"xT{b}")
            t1pk = bpool.tile([C, 18], BF16, tag=f"t1pk{b}")
            for ci, (c0, cn) in enumerate(CHUNKS512):
                psx = ps_wide.tile([C, 512], F32, tag="w")
                pxv = psx[:].bitcast(BF16)
                g = cn // 128
                for j in range(g):
                    col = c0 + j * 128
                    nc.tensor.transpose(
                        pxv[:, j * 128 : (j + 1) * 128],
                        x_l[b][:, col : col + 128],
                        ident_bf,
                    )
                nc.scalar.copy(xT[:, c0 : c0 + cn], pxv[:, :cn])
                nc.vector.reduce_max(
                    t1pk[:, 4 * ci : 4 * ci + g],
                    pxv[:, :cn].rearrange("p (g q) -> p g q", q=128),
                    axis=mybir.AxisListType.X,
                )
            xT_l.append(xT)
            t1pk_l.append(t1pk)
            pst = ps_cc.tile([C, C], F32, tag="cc")
            pstv = pst[:].bitcast(BF16)
            nc.tensor.transpose(pstv[0:18, 0:128], t1pk[:], ident_bf)
            t1row = bpool.tile([18, C], BF16, tag=f"t1row{b}")
            nc.vector.tensor_copy(t1row[:], pstv[0:18, 0:128])
            t1row_l.append(t1row)
            if b == 0:
                # V gets min-b0 between b0's and b1's reduces
                tmin0 = bpool.tile([C, HW], BF16, tag="tmin0")
                emit_min(0, tmin0)
                tmin_l[0] = tmin0
                # SP: chain-b0 A+K right away
                nc.sync.dma_start(t1flat_l[0][:, 96 : 96 + HW], t1row[:])
                for k in range(3):
                    k_dma(nc.sync, 0, k)

        # chain b1: A+K on ACT (after the xT drains in the ACT queue)
        nc.scalar.dma_start(t1flat_l[1][:, 96 : 96 + HW], t1row_l[1][:])
        for k in range(3):
            k_dma(nc.scalar, 1, k)
        # J hops on SP, b0 then b1
        for b in range(NB):
            for j in range(5):
                j_dma(nc.sync, b, j)

        # ---- V: min-b1, mult-b0, mult-b1; PE: t5T transposes -------------
        tmin1 = bpool.tile([C, HW], BF16, tag="tmin1")
        emit_min(1, tmin1)
        tmin_l[1] = tmin1

        t5T_l = []
        for b in range(NB):
            t5 = bpool.tile([C, HW], BF16, tag=f"t5{b}")
            for c0, cn in ((0, 1152), (1152, 1152)):
                nc.vector.tensor_tensor(
                    t5[:, c0 : c0 + cn],
                    tmin_l[b][:, c0 : c0 + cn],
                    p5_sb[:, c0 : c0 + cn],
                    mmul,
                )
            t5T = bpool.tile([C, HW], BF16, tag=f"t5T{b}")
            for c0, cn in CHUNKS512:
                psx = ps_wide.tile([C, 512], F32, tag="w")
                pxv = psx[:].bitcast(BF16)
                for j in range(cn // 128):
                    col = c0 + j * 128
                    nc.tensor.transpose(
                        pxv[:, j * 128 : (j + 1) * 128],
                        t5[:, col : col + 128],
                        ident_bf,
                    )
                nc.scalar.copy(t5T[:, c0 : c0 + cn], pxv[:, :cn])
            t5T_l.append(t5T)

        # ---- PE filler while the chains drain ----------------------------
        dscr = ps_wide.tile([C, 512], F32, tag="w")
        for i in range(NDUMMY):
            nc.tensor.transpose(
                dscr[:].bitcast(BF16)[:, (i % 4) * 128 : (i % 4) * 128 + 128],
                ident_bf[:],
                ident_bf,
            )

        # ---- t11T + max -> t12T with lagged G matmuls --------------------
        psG_l = []
        for b in range(NB):
            t12T = bpool.tile([C, HW], BF16, tag=f"t12T{b}")
            psG = ps_cc.tile([C, C], F32, tag="cc")
            psG_l.append(psG)

            def g_mms(ci):
                c0x, cnx = CHUNKS512[ci]
                for j in range(cnx // 128):
                    i = 4 * ci + j
                    nc.tensor.matmul(
                        psG[:],
                        t5T_l[b][:, i * 128 : (i + 1) * 128],
                        t12T[:, i * 128 : (i + 1) * 128],
                        start=(i == 0),
                        stop=(i == 17),
                    )

            for ci, (c0, cn) in enumerate(CHUNKS512):
                ps11 = ps_wide.tile([C, 512], F32, tag="w")
                g = cn // 128
                for j in range(g):
                    col = c0 + j * 128
                    nc.tensor.matmul(
                        ps11[:, j * 128 : (j + 1) * 128],
                        shifted_l[b][:, col : col + 128],
                        K_effT[:],
                        start=True,
                        stop=True,
                    )
                nc.vector.tensor_tensor(
                    t12T[:, c0 : c0 + cn],
                    xT_l[b][:, c0 : c0 + cn],
                    ps11[:, :cn],
                    mmax,
                )
                if ci > 0:
                    g_mms(ci - 1)
            g_mms(len(CHUNKS512) - 1)

        # ---- Gs -> B' -> Bs -> out ---------------------------------------
        for b in range(NB):
            Gs = bpool.tile([C, C], BF16, tag=f"Gs{b}")
            nc.scalar.copy(Gs[:], psG_l[b][:])
            psB = ps_cc.tile([C, C], F32, tag="cc")
            nc.tensor.matmul(psB[:], w6T[:], Gs[:], start=True, stop=True)
            Bs = bpool.tile([C, C], BF16, tag=f"Bs{b}")
            nc.scalar.mul(Bs[:], psB[:], SCALE)

            out_sb = bpool.tile([C, HW], BF16, tag=f"osb{b}")
            out_ap = out_d.ap()[b]
            for ci, (c0, cn) in enumerate(CHUNKS512):
                pso = ps_wide.tile([C, 512], F32, tag="w")
                nc.tensor.matmul(
                    pso[:, :cn], Bs[:], x_l[b][:, c0 : c0 + cn], start=True, stop=True
                )
                if ci % 2 == 0:
                    nc.vector.tensor_copy(out_sb[:, c0 : c0 + cn], pso[:, :cn])
                else:
                    nc.scalar.copy(out_sb[:, c0 : c0 + cn], pso[:, :cn])
                if ci == 2:
                    nc.sync.dma_start(out_ap[:, 0:1152], out_sb[:, 0:1152])
            nc.sync.dma_start(out_ap[:, 1152:HW], out_sb[:, 1152:HW])


def build_bass():
    nc = bacc.Bacc("TRN2", target_bir_lowering=False, debug=False, num_devices=NCORES)
    x_d = nc.dram_tensor("x", [NB, C, HW], BF16, kind="ExternalInput")
    p5_d = nc.dram_tensor("p5b", [C, HW], BF16, kind="ExternalInput")
    w6T_d = nc.dram_tensor("w6T", [C, C], BF16, kind="ExternalInput")
    keff_d = nc.dram_tensor("KeffT", [15, C], BF16, kind="ExternalInput")
    ident_d = nc.dram_tensor("ident", [128, 128], BF16, kind="ExternalInput")
    out_d = nc.dram_tensor("out", [NB, C, HW], BF16, kind="ExternalOutput")
    with tile.TileContext(nc) as tc:
        build_kernel(tc, out_d, x_d, p5_d, w6T_d, keff_d, ident_d)
    nc.compile()
    return nc


_NC_CACHE = {}


def _get_nc():
    if "nc" not in _NC_CACHE:
        _NC_CACHE["nc"] = build_bass()
    return _NC_CACHE["nc"]


def _host_weights(p5_w, w6, w8, w10):
    p5b = np.asarray(p5_w, np.float32).reshape(C, HW).astype(BF16NP)
    w6T = np.ascontiguousarray(np.asarray(w6, np.float32).T).astype(BF16NP)
    w8v = np.asarray(w8, np.float32)[:, 0, 0, :]
    w10v = np.asarray(w10, np.float32)
    w10r = np.roll(w10v, 1, axis=0).reshape(C, C // 2, 3)
    keff = np.einsum("mj,omk->jko", w8v, w10r).reshape(15, C)
    return p5b, w6T, keff.astype(BF16NP), np.eye(128, dtype=BF16NP)


def kernel(x, p5_w, w6, w8, w10, trace=False, trace_kwargs=None):
    x_bf = (
        np.ascontiguousarray(np.asarray(x, dtype=np.float32))
        .reshape(N, C, HW)
        .astype(BF16NP)
    )
    p5b, w6T, keff, ident = _host_weights(p5_w, w6, w8, w10)
    nc = _get_nc()
    in_maps = []
    for core in range(NCORES):
        in_maps.append(
            {
                "x": x_bf[core * NB : (core + 1) * NB],
                "p5b": p5b,
                "w6T": w6T,
                "KeffT": keff,
                "ident": ident,
            }
        )
    res = run_bass_kernel_spmd(
        nc,
        in_maps,
        list(range(NCORES)),
        trace=trace,
        **(trace_kwargs or {}),
    )
    out = np.concatenate(
        [res.results[i]["out"].astype(np.float32) for i in range(NCORES)], axis=0
    ).reshape(N, C, H, W)
    if trace:
        return out, res
    return out
